# revision 6
# baseline (speedup 1.0000x reference)
"""CLIP (NT-Xent style) loss on 8 Trainium2 NeuronCores.

Strategy: data-parallel over the batch. Core c receives rows
[c*1024, (c+1)*1024) of z_i (strip), ALL of z_j, and the matching strip of
z_j (for the exact-diagonal computation). Each core computes its
1024 x 8192 strip of exp(logits) implicitly and reduces it on-chip to:
  - rowsum[1024]:  sum_j exp(2 * <zi_n[i], zj_n[j]>)    (full row -> row LSE)
  - colsum[8192]:  sum_{i in strip} exp(...)            (partial; host sums)
  - diag[1024]:    2 * <zi_n[i], zj_n[i]>  in f32       (exact diagonal)
Host combines in float64: loss = 0.5*(mean LSE_r + mean LSE_c) - mean diag.

Numerics: logits are bounded in [-2, 2] (cosine / 0.5), so exp needs no max
subtraction. The big matmul runs in bf16 (inputs rounded to bf16, fp32 PSUM
accumulation); averaging over 8192-term logsumexps makes the resulting loss
error ~4e-7 relative (verified against f64 on CPU).
"""

import numpy as np

B = 8192
D = 1024
NCORES = 8
M = B // NCORES          # 1024 rows of z_i per core
NT_I = M // 128          # 8 partition-tiles of zi
NT_J = B // 128          # 64 partition-tiles of zj
DC = D // 128            # 8 contraction chunks
JBLK = 8                 # zj tiles per pipeline block
NBLK = NT_J // JBLK      # 8 blocks
LN2 = 0.6931471805599453

_CACHE = {}


def _build_nc(nblk=NBLK):
    import sys
    try:
        import concourse.bass  # noqa: F401
    except ImportError:
        sys.path.insert(0, "/opt/trn_rl_repo")
    import concourse.mybir as mybir
    import concourse.tile as tile
    from concourse import bacc

    f32 = mybir.dt.float32
    bf16 = mybir.dt.bfloat16
    AF = mybir.ActivationFunctionType
    OP = mybir.AluOpType

    nc = bacc.Bacc("TRN2", target_bir_lowering=False, debug=False,
                   num_devices=NCORES)

    zi = nc.dram_tensor("zi", [M, D], f32, kind="ExternalInput")
    zj = nc.dram_tensor("zj", [B, D], f32, kind="ExternalInput")
    zjd = nc.dram_tensor("zjd", [M, D], f32, kind="ExternalInput")
    rowsum_out = nc.dram_tensor("rowsum", [1, M], f32, kind="ExternalOutput")
    colsum_out = nc.dram_tensor("colsum", [128, NT_J], f32, kind="ExternalOutput")
    diag_out = nc.dram_tensor("diag", [128, NT_I], f32, kind="ExternalOutput")

    with tile.TileContext(nc) as tc:
        with (
            tc.tile_pool(name="pers", bufs=1) as pers,
            tc.tile_pool(name="zix", bufs=NT_I) as zix_pool,
            tc.tile_pool(name="xin", bufs=4) as xpool,
            tc.tile_pool(name="hi", bufs=3) as hipool,
            tc.tile_pool(name="scr", bufs=2) as scrpool,
            tc.tile_pool(name="sml", bufs=2) as smlpool,
            tc.tile_pool(name="zjt", bufs=3) as zjt_pool,
            tc.tile_pool(name="exp", bufs=6) as exp_pool,
            tc.tile_pool(name="cacc", bufs=4) as cacc_pool,
            tc.tile_pool(name="psmain", bufs=2, space="PSUM") as psum_main,
            tc.tile_pool(name="psrow", bufs=1, space="PSUM") as psum_row,
            tc.tile_pool(name="dram", bufs=3, space="DRAM") as dram_pool,
            tc.tile_pool(name="dramp", bufs=1, space="DRAM") as dram_pers,
        ):
            # persistent tiles
            ones = pers.tile([128, 1], bf16, tag="ones")
            nc.vector.memset(ones, 1.0)
            ln2b = pers.tile([128, 1], f32, tag="ln2b")
            nc.vector.memset(ln2b, LN2)
            stats_i = pers.tile([128, NT_I], f32, tag="stats_i")
            stats_jd = pers.tile([128, NT_I], f32, tag="stats_jd")
            rdots = pers.tile([128, NT_I], f32, tag="rdots")
            rn_i = pers.tile([128, NT_I], f32, tag="rn_i")
            rn_jd2 = pers.tile([128, NT_I], f32, tag="rn_jd2")
            stats_j = pers.tile([128, NT_J], f32, tag="stats_j")
            scale2_j = pers.tile([128, NT_J], f32, tag="scale2_j")
            diag_sb = pers.tile([128, NT_I], f32, tag="diag_sb")
            colsum_sb = pers.tile([128, NT_J], f32, tag="colsum_sb")
            ziT = pers.tile([128, DC, M], bf16, tag="ziT")
            zi_hi_dram = dram_pers.tile([M, D], bf16, tag="zi_hi_dram")

            # ---- zi + zj-diag pass: stats + raw diag dots ----
            zi_tiles = []
            for t in range(NT_I):
                xi = zix_pool.tile([128, D], f32, tag="zix")
                nc.sync.dma_start(xi[:], zi[t * 128:(t + 1) * 128, :])
                zi_tiles.append(xi)
                xd = xpool.tile([128, D], f32, tag="xd")
                nc.sync.dma_start(xd[:], zjd[t * 128:(t + 1) * 128, :])
                s = scrpool.tile([128, D], f32, tag="scr")
                nc.scalar.activation(s[:], xi[:], AF.Square,
                                     accum_out=stats_i[:, t:t + 1])
                s = scrpool.tile([128, D], f32, tag="scr")
                nc.scalar.activation(s[:], xd[:], AF.Square,
                                     accum_out=stats_jd[:, t:t + 1])
                s = scrpool.tile([128, D], f32, tag="scr")
                nc.vector.tensor_mul(s[:], xi[:], xd[:])
                nc.vector.reduce_sum(rdots[:, t:t + 1], s[:],
                                     axis=mybir.AxisListType.X)

            # rn_i = exp(-0.5 ln stats_i);  rn_jd2 = exp(-0.5 ln stats_jd + ln2)
            lt = smlpool.tile([128, NT_I], f32, tag="ln_i")
            nc.scalar.activation(lt[:], stats_i[:], AF.Ln)
            nc.scalar.activation(rn_i[:], lt[:], AF.Exp, scale=-0.5)
            lt = smlpool.tile([128, NT_I], f32, tag="ln_i")
            nc.scalar.activation(lt[:], stats_jd[:], AF.Ln)
            nc.scalar.activation(rn_jd2[:], lt[:], AF.Exp, bias=ln2b[:], scale=-0.5)

            # diag = rdots * rn_i * rn_jd2   (rn_jd2 = 2/||zj_d||)
            dtmp = smlpool.tile([128, NT_I], f32, tag="dtmp")
            nc.vector.tensor_mul(dtmp[:], rdots[:], rn_i[:])
            nc.vector.tensor_mul(diag_sb[:], dtmp[:], rn_jd2[:])
            nc.sync.dma_start(diag_out[:], diag_sb[:])

            # zi: scale+cast to bf16, bounce через DRAM, transpose to [D, M]
            for t in range(NT_I):
                hi = hipool.tile([128, D], bf16, tag="zihi")
                nc.scalar.activation(hi[:], zi_tiles[t][:], AF.Copy,
                                     bias=0.0, scale=rn_i[:, t:t + 1])
                nc.sync.dma_start(zi_hi_dram[t * 128:(t + 1) * 128, :], hi[:])
            for d in range(DC):
                nc.sync.dma_start_transpose(
                    ziT[:, d, :], zi_hi_dram[:, d * 128:(d + 1) * 128])

            # rowsum accumulator: one PSUM tile [1, 1024] spanning 2 banks,
            # accumulated by ones-matmuls across all 64 j-tiles.
            rowsum_ps = psum_row.tile([1, M], f32, tag="rowsum_ps")

            prev = None  # (jt, [exp tiles]) pipelined rowsum matmuls

            def emit_rowsum(prev):
                jt, exps = prev
                for ic in range(2):
                    nc.tensor.matmul(
                        rowsum_ps[0:1, ic * 512:(ic + 1) * 512],
                        ones[:], exps[ic][:],
                        start=(jt == 0), stop=(jt == nblk * JBLK - 1))

            # ---- main pipeline over 8 blocks of 8 j-tiles ----
            for blk in range(nblk):
                blk_dram = dram_pool.tile([JBLK * 128, D], bf16, tag="zjhi")
                for tt in range(JBLK):
                    jt = blk * JBLK + tt
                    x = xpool.tile([128, D], f32, tag="zjx")
                    nc.sync.dma_start(x[:], zj[jt * 128:(jt + 1) * 128, :])
                    s = scrpool.tile([128, D], f32, tag="scr")
                    nc.scalar.activation(s[:], x[:], AF.Square,
                                         accum_out=stats_j[:, jt:jt + 1])
                    hi = hipool.tile([128, D], bf16, tag="zjhi_sb")
                    nc.vector.tensor_copy(hi[:], x[:])
                    nc.sync.dma_start(blk_dram[tt * 128:(tt + 1) * 128, :], hi[:])

                # scale2_j[:, blk] = exp(-0.5 ln stats + ln2) = 2/||zj||
                sl = slice(blk * JBLK, (blk + 1) * JBLK)
                lt = smlpool.tile([128, JBLK], f32, tag="ln_j")
                nc.scalar.activation(lt[:], stats_j[:, sl], AF.Ln)
                nc.scalar.activation(scale2_j[:, sl], lt[:], AF.Exp,
                                     bias=ln2b[:], scale=-0.5)

                zjt = zjt_pool.tile([128, DC, JBLK * 128], bf16, tag="zjt")
                for d in range(DC):
                    nc.sync.dma_start_transpose(
                        zjt[:, d, :], blk_dram[:, d * 128:(d + 1) * 128])

                for tt in range(JBLK):
                    jt = blk * JBLK + tt
                    ps = psum_main.tile([128, M], f32, tag="ps")
                    for d in range(DC):
                        lhsT = zjt[:, d, tt * 128:(tt + 1) * 128]
                        nc.tensor.matmul(ps[:, 0:512], lhsT,
                                         ziT[:, d, 0:512],
                                         start=(d == 0), stop=(d == DC - 1))
                        nc.tensor.matmul(ps[:, 512:1024], lhsT,
                                         ziT[:, d, 512:1024],
                                         start=(d == 0), stop=(d == DC - 1))
                    cacc = cacc_pool.tile([128, 2], f32, tag="cacc")
                    exps = []
                    for ic in range(2):
                        ex = exp_pool.tile([128, 512], bf16, tag="exp")
                        nc.scalar.activation(
                            ex[:], ps[:, ic * 512:(ic + 1) * 512], AF.Exp,
                            scale=scale2_j[:, jt:jt + 1],
                            accum_out=cacc[:, ic:ic + 1])
                        exps.append(ex)
                    if prev is not None:
                        emit_rowsum(prev)
                    prev = (jt, exps)
                    nc.vector.tensor_add(colsum_sb[:, jt:jt + 1],
                                         cacc[:, 0:1], cacc[:, 1:2])

            emit_rowsum(prev)

            rs_sb = pers.tile([1, M], f32, tag="rs_sb")
            nc.vector.tensor_copy(rs_sb[:], rowsum_ps[:])
            nc.sync.dma_start(rowsum_out[:], rs_sb[:])
            nc.sync.dma_start(colsum_out[:], colsum_sb[:])

    nc.compile()
    return nc


def _get_nc():
    if "nc" not in _CACHE:
        _CACHE["nc"] = _build_nc()
    return _CACHE["nc"]


def kernel(z_i: np.ndarray, z_j: np.ndarray) -> np.ndarray:
    from concourse import bass_utils

    nc = _get_nc()
    z_i = np.ascontiguousarray(z_i, dtype=np.float32)
    z_j = np.ascontiguousarray(z_j, dtype=np.float32)
    in_maps = []
    for c in range(NCORES):
        sl = slice(c * M, (c + 1) * M)
        in_maps.append({
            "zi": np.ascontiguousarray(z_i[sl]),
            "zj": z_j,
            "zjd": np.ascontiguousarray(z_j[sl]),
        })
    res = bass_utils.run_bass_kernel_spmd(nc, in_maps,
                                          core_ids=list(range(NCORES)))
    return _combine([r for r in res.results])


def _combine(results) -> np.ndarray:
    rowsum_all = np.empty((NCORES, M), np.float64)
    diag_all = np.empty((NCORES, M), np.float64)
    colsum_tot = np.zeros(B, np.float64)
    for c, r in enumerate(results):
        rowsum_all[c] = r["rowsum"][0].astype(np.float64)
        # colsum[p, jt] -> j = jt*128 + p
        colsum_tot += r["colsum"].astype(np.float64).T.reshape(B)
        # diag[p, t] -> i = t*128 + p
        diag_all[c] = r["diag"].astype(np.float64).T.reshape(M)
    lse_r = np.log(rowsum_all).mean()
    lse_c = np.log(colsum_tot).mean()
    loss = 0.5 * (lse_r + lse_c) - diag_all.mean()
    return np.float32(loss)
